# revision 13
# baseline (speedup 1.0000x reference)
"""Fused attention kernel for TRN2, 8 NeuronCores, data-parallel over batch.

Problem: q = target @ Wq.T + bq ; k = non_target @ Wk.T + bk ;
         v = non_target @ Wv.T + bv ; out = softmax(q k^T) v
Shapes: target/non_target [8, 2048, 1024], W* [1024, 1024], b* [1024].

Math (per batch, one core):
  softmax(q k^T) is row-shift invariant, so kv-constant terms drop:
    S' = T M N^T + 1 w^T,  M = Wq^T Wk,  w = N (Wk^T bq)   (bk drops out)
  Work transposed: S'^T = N G',  G'[d',q] = sum_d M[d,d'] T^T[d,q] + y[d'],
  y = Wk^T bq.  P^T = exp(S'^T) unnormalized, r[q] = sum_kv P^T[kv,q],
    O[q,e] = ( sum_d Z^T[d,q] Wv^T[d,e] ) / r[q] + bv[e]
    Z^T[d,q] = sum_kv N[kv,d] P^T[kv,q]

Scores path (S') stays fp32r end-to-end.  The value path (P, N-for-Z, Z,
Wv^T) is bf16: rel tolerance is 2e-2 and bf16 rounding there only perturbs
the weighted average of v, not the softmax logits.  (The PE requires both
matmul operands f32/f32r or both 16-bit, so the paths never mix dtypes.)

Residency: N^T (f32r, 8MB), N (bf16, 4MB), M (f32r, 4MB), Wv^T (bf16, 2MB)
all live in SBUF, so steady-state HBM traffic is only T in / O out
(2MB/chunk).  N and Wv land once; N^T / bf16-N builds are pipelined into
chunk 0's kv loop, Wv^T builds into chunk 0's tail.  M is built with
eb-outer accumulation across all 8 PSUM banks so its matmuls trail the
per-half-block Wq/Wk DMAs instead of waiting for the full weight load.
"""

import numpy as np

import concourse.bass as bass
import concourse.mybir as mybir
import concourse.tile as tile
from concourse import bacc
from concourse.bass_utils import run_bass_kernel_spmd
from concourse.masks import make_identity

F32 = mybir.dt.float32
F32R = mybir.dt.float32r
BF16 = mybir.dt.bfloat16

B, SQ, SKV, D = 8, 2048, 2048, 1024
P = 128
QC = 256                 # q-chunk size
NCHUNK = SQ // QC        # 8
DB = D // P              # 8 d-blocks
KVB = SKV // P           # 16 kv-blocks
NPAIR = KVB // 2         # 8 kv-pairs per chunk
NCORES = 8

_CACHE = {}


def _build():
    nc = bacc.Bacc()
    tgt = nc.declare_dram_parameter("target", [SQ, D], F32R, isOutput=False)
    ntg = nc.declare_dram_parameter("non_target", [SKV, D], F32R, isOutput=False)
    wqp = nc.declare_dram_parameter("Wq", [D, D], F32R, isOutput=False)
    wkp = nc.declare_dram_parameter("Wk", [D, D], F32R, isOutput=False)
    wvp = nc.declare_dram_parameter("Wv", [D, D], F32R, isOutput=False)
    bqp = nc.declare_dram_parameter("bq", [D], F32, isOutput=False)
    bvp = nc.declare_dram_parameter("bv", [D], F32, isOutput=False)
    outp = nc.declare_dram_parameter("out", [SQ, D], F32, isOutput=True)
    with tile.TileContext(nc) as tc:
        _emit(nc, tc, tgt, ntg, wqp, wkp, wvp, bqp, bvp, outp)
    nc.compile()
    return nc


def _emit(nc, tc, tgt, ntg, wqp, wkp, wvp, bqp, bvp, outp):
    import contextlib
    ctx = contextlib.ExitStack()
    with ctx:
        # ---- small residents ----
        R = ctx.enter_context(tc.tile_pool(name="resident", bufs=1))
        identF = R.tile([P, P], F32)
        make_identity(nc, identF)
        identR = R.tile([P, P], F32R)
        nc.vector.tensor_copy(identR, identF)
        identB = R.tile([P, P], BF16)
        nc.vector.tensor_copy(identB, identF)
        Mt = R.tile([P, DB, D], F32R)       # M: [d in block, d-block, d']
        y_col = R.tile([P, DB], F32)        # y: [d' in block, d'-block]
        w_col = R.tile([P, KVB], F32)       # w = N y: [kv in block, kv-block]
        bq_col = R.tile([P, DB], F32)       # bq: [e in block, e-block]
        bv_bc = R.tile([P, D], F32)         # bv broadcast to 128 partitions
        nc.sync.dma_start(bq_col, bqp[:].rearrange("(b p) -> p b", p=P))
        bv_bcast_ap = bass.AP(
            tensor=bvp[:].tensor, offset=0,
            ap=[[0, P], [1, D]],
        )
        nc.gpsimd.dma_start(out=bv_bc, in_=bv_bcast_ap)

        # ---- N^T resident (f32r, scores path) ----
        Rnt = ctx.enter_context(tc.tile_pool(name="rnt", bufs=1))
        NT = Rnt.tile([P, DB, SKV], F32R)   # N^T: [d' in block, d'-block, kv]

        # ---- M = Wq^T Wk, eb-outer over per-half-block weight DMAs ----
        # Wq/Wk land as [128, 512] half-row-block DMAs; accumulation groups
        # live across all 8 PSUM banks so each eb's matmuls fire as soon as
        # that eb's halves have landed.  Group order within a pass is
        # interleaved (db0ch0, db1ch0, db0ch1, db1ch1, db2ch0, ...) to match
        # the half-arrival order.
        with tc.tile_pool(name="wtmp", bufs=1) as W, \
             tc.tile_pool(name="pp8", bufs=1, space="PSUM") as pp8:
            wq_eb = [W.tile([P, D], F32R, name=f"wq{eb}") for eb in range(DB)]
            wk_eb = [W.tile([P, D], F32R, name=f"wk{eb}") for eb in range(DB)]
            for eb in range(DB):
                nc.sync.dma_start(wq_eb[eb][:, 0:512],
                                  wqp[eb * P:(eb + 1) * P, 0:512])
                nc.scalar.dma_start(wk_eb[eb][:, 0:512],
                                    wkp[eb * P:(eb + 1) * P, 0:512])
                nc.sync.dma_start(wq_eb[eb][:, 512:D],
                                  wqp[eb * P:(eb + 1) * P, 512:D])
                nc.scalar.dma_start(wk_eb[eb][:, 512:D],
                                    wkp[eb * P:(eb + 1) * P, 512:D])
            GORDER = [(0, 0), (1, 0), (0, 1), (1, 1), (2, 0), (3, 0), (2, 1), (3, 1)]
            for half in range(2):
                mps = [pp8.tile([P, 512], F32, tag=f"m{g}", name=f"mps{g}")
                       for g in range(8)]
                for eb in range(DB):
                    for g, (dbo, ch) in enumerate(GORDER):
                        db = 4 * half + dbo
                        nc.tensor.matmul(
                            mps[g],
                            wq_eb[eb][:, db * P:(db + 1) * P],
                            wk_eb[eb][:, ch * 512:(ch + 1) * 512],
                            start=(eb == 0), stop=(eb == DB - 1),
                        )
                for g, (dbo, ch) in enumerate(GORDER):
                    db = 4 * half + dbo
                    if g % 2 == 0:
                        nc.vector.tensor_copy(Mt[:, db, ch * 512:(ch + 1) * 512],
                                              mps[g])
                    else:
                        nc.scalar.activation(Mt[:, db, ch * 512:(ch + 1) * 512],
                                             mps[g],
                                             mybir.ActivationFunctionType.Copy)
            # y = Wk^T bq (tiny matmuls, psum bank reuse)
            for ob in range(DB):
                yp = pp8.tile([P, 512], F32, tag="m0", name="yp")
                for eb in range(DB):
                    nc.tensor.matmul(
                        yp[:, 0:1],
                        wk_eb[eb][:, ob * P:(ob + 1) * P].bitcast(F32),
                        bq_col[:, eb:eb + 1],
                        start=(eb == 0), stop=(eb == DB - 1),
                    )
                nc.vector.tensor_copy(y_col[:, ob:ob + 1], yp[:, 0:1])

        # ---- pools that live from here on (fit in space freed by wtmp) ----
        Rnz = ctx.enter_context(tc.tile_pool(name="rnz", bufs=1))
        Nz = Rnz.tile([P, KVB, D], BF16)    # N: [kv in block, kv-block, d]
        Rwv = ctx.enter_context(tc.tile_pool(name="rwv", bufs=1))
        WvT = Rwv.tile([P, DB, D], BF16)    # Wv^T: [d in block, d-block, e]

        nld = ctx.enter_context(tc.tile_pool(name="nld", bufs=3))
        wcv = ctx.enter_context(tc.tile_pool(name="wcv", bufs=2))

        # ---- main-loop pools ----
        sp = ctx.enter_context(tc.tile_pool(name="sp", bufs=2, space="PSUM"))
        zpp = ctx.enter_context(tc.tile_pool(name="zp", bufs=1, space="PSUM"))
        mp = ctx.enter_context(tc.tile_pool(name="mp", bufs=2, space="PSUM"))
        tld = ctx.enter_context(tc.tile_pool(name="tld", bufs=2))
        chk = ctx.enter_context(tc.tile_pool(name="chk", bufs=1))
        ptp = ctx.enter_context(tc.tile_pool(name="ptp", bufs=3))
        osb = ctx.enter_context(tc.tile_pool(name="osb", bufs=2))
        smal = ctx.enter_context(tc.tile_pool(name="smal", bufs=4))

        tl_tiles = {}

        def emit_tload(c):
            q0 = c * QC
            for qb in range(QC // P):
                tl = tld.tile([P, D], F32R, tag="tload", name="tl")
                nc.sync.dma_start(tl, tgt[q0 + qb * P:q0 + (qb + 1) * P, :])
                tl_tiles[(c, qb)] = tl

        ring_tiles = {}

        def emit_nload(j):
            nl = nld.tile([P, D], F32R, tag="nld", name="nl")
            eng = nc.sync if j % 2 == 0 else nc.scalar
            eng.dma_start(nl, ntg[j * P:(j + 1) * P, :])
            ring_tiles[("n", j)] = nl

        def emit_wvload(eb):
            wv = nld.tile([P, D], F32R, tag="nld", name="wv")
            eng = nc.sync if eb % 2 == 0 else nc.scalar
            eng.dma_start(wv, wvp[eb * P:(eb + 1) * P, :])
            ring_tiles[("wv", eb)] = wv

        def emit_nt_build(j):
            """N^T tile j (PE transposes via mp psum) + bf16 N_z copy."""
            nl = ring_tiles.pop(("n", j))
            for g in range(2):
                tp = mp.tile([P, 512], F32R, tag="mp", name="tp")
                tpv = tp.rearrange("p (k c) -> p k c", k=4)
                for k in range(4):
                    db = 4 * g + k
                    nc.tensor.transpose(tpv[:, k, :], nl[:, db * P:(db + 1) * P],
                                        identR)
                dst = NT[:, 4 * g:4 * g + 4, j * P:(j + 1) * P]
                if g == 0:
                    nc.vector.tensor_copy(dst, tpv)
                else:
                    nc.scalar.activation(dst, tpv,
                                         mybir.ActivationFunctionType.Copy)
            nc.gpsimd.tensor_copy(Nz[:, j, :], nl)
            # w[kv] = sum_d' N[kv,d'] y[d'] — the per-kv score shift from bq,
            # fused later into exp's bias (frees the per-chunk Gp bias adds)
            wp = mp.tile([P, 512], F32, tag="mp", name="wp")
            for ob in range(DB):
                nc.tensor.matmul(
                    wp[:, 0:1],
                    NT[:, ob, j * P:(j + 1) * P].bitcast(F32),
                    y_col[:, ob:ob + 1],
                    start=(ob == 0), stop=(ob == DB - 1),
                )
            nc.vector.tensor_copy(w_col[:, j:j + 1], wp[:, 0:1])

        def emit_wvt_build(eb):
            """Wv^T columns for e-block eb (f32r transposes; the PSUM->SBUF
            drain copies convert to bf16)."""
            wv = ring_tiles.pop(("wv", eb))
            for g in range(2):
                tp = mp.tile([P, 512], F32R, tag="mp", name="tpv")
                tpv = tp.rearrange("p (k c) -> p k c", k=4)
                for k in range(4):
                    db = 4 * g + k
                    nc.tensor.transpose(tpv[:, k, :], wv[:, db * P:(db + 1) * P],
                                        identR)
                dst = WvT[:, 4 * g:4 * g + 4, eb * P:(eb + 1) * P]
                if g == 0:
                    nc.vector.tensor_copy(dst, tpv)
                else:
                    nc.scalar.activation(dst, tpv,
                                         mybir.ActivationFunctionType.Copy)

        def emit_tt(c):
            """T^T for chunk c from prefetched tl tiles."""
            TT = chk.tile([P, DB, QC], F32R, tag="tt", name="TT")
            for qb in range(QC // P):
                tl = tl_tiles.pop((c, qb))
                for g in range(2):
                    tp = mp.tile([P, 512], F32R, tag="mp", name="tp")
                    tpv = tp.rearrange("p (k c) -> p k c", k=4)
                    for k in range(4):
                        db = 4 * g + k
                        nc.tensor.transpose(tpv[:, k, :],
                                            tl[:, db * P:(db + 1) * P], identR)
                    dst = TT[:, 4 * g:4 * g + 4, qb * P:(qb + 1) * P]
                    if g == 0:
                        nc.vector.tensor_copy(dst, tpv)
                    else:
                        nc.scalar.activation(dst, tpv,
                                             mybir.ActivationFunctionType.Copy)
            return TT

        def emit_gp(TT):
            Gp = chk.tile([P, DB, QC], F32R, tag="gp", name="Gp")
            for ob in range(DB):
                gp_ps = mp.tile([P, 512], F32, tag="mp", name="gp_ps")
                for db in range(DB):
                    nc.tensor.matmul(
                        gp_ps[:, 0:QC],
                        Mt[:, db, ob * P:(ob + 1) * P],
                        TT[:, db, :],
                        start=(db == 0), stop=(db == DB - 1),
                    )
                if ob % 2 == 0:
                    nc.vector.tensor_copy(Gp[:, ob, :], gp_ps[:, 0:QC])
                else:
                    nc.scalar.activation(Gp[:, ob, :], gp_ps[:, 0:QC],
                                         mybir.ActivationFunctionType.Copy)
            return Gp

        # DMA order from here: T chunk 0, N tiles, Wv blocks (ring-throttled)
        emit_tload(0)
        for j in range(KVB):
            emit_nload(j)
        for eb in range(DB):
            emit_wvload(eb)

        # NT tiles 0,1 before chunk 0's first S' pair; rest pipeline in-loop
        emit_nt_build(0)
        emit_nt_build(1)
        TT = emit_tt(0)
        Gp = emit_gp(TT)

        for c in range(NCHUNK):
            q0 = c * QC
            if c + 1 < NCHUNK:
                emit_tload(c + 1)   # prefetch next chunk's T now
            # ---- kv loop: S' -> exp -> racc ; Z pipelined one pair behind ----
            zp = zpp.tile([P, DB, QC], F32, name="zp")
            racc2 = chk.tile([P, 2, QC], F32, tag="racc2", name="racc2")
            racc = chk.tile([P, QC], F32R, tag="racc", name="racc")
            pts = {}
            for jj in range(NPAIR + 1):
                if jj < NPAIR:
                    spt = sp.tile([P, 2, QC], F32, name="spt")
                    for h in range(2):
                        j = 2 * jj + h
                        for ob in range(DB):
                            nc.tensor.matmul(
                                spt[:, h, :],
                                NT[:, ob, j * P:(j + 1) * P],
                                Gp[:, ob, :],
                                start=(ob == 0), stop=(ob == DB - 1),
                            )
                    pt = ptp.tile([P, 2, QC], BF16, tag="pt", name="pt")
                    for h in range(2):
                        j = 2 * jj + h
                        nc.scalar.activation(pt[:, h, :], spt[:, h, :],
                                             mybir.ActivationFunctionType.Exp,
                                             bias=w_col[:, j:j + 1])
                    pts[jj] = pt
                    if jj == 0:
                        nc.gpsimd.tensor_copy(racc2, pt)
                    else:
                        nc.gpsimd.tensor_add(racc2, racc2, pt)
                    if c == 0:
                        # pipeline the one-time N^T builds into chunk 0's slack
                        for j in (2 * jj + 2, 2 * jj + 3):
                            if 2 <= j < KVB:
                                emit_nt_build(j)
                if jj > 0:
                    zjj = jj - 1
                    pt = pts.pop(zjj)
                    for h in range(2):
                        j = 2 * zjj + h
                        for db in range(DB):
                            # start only on the even db of each 2KB PSUM zero
                            # region: its start marks the whole region, and
                            # the odd db's first write lands as overwrite
                            # (a second start would re-mark the even db's
                            # fresh j=0 data as pending-zero and lose it)
                            nc.tensor.matmul(
                                zp[:, db, :],
                                Nz[:, j, db * P:(db + 1) * P],
                                pt[:, h, :],
                                start=(j == 0 and db % 2 == 0),
                                stop=(j == KVB - 1),
                                skip_group_check=True,
                            )

            # ---- next chunk's TT transposes first: their DVE/Act drains
            # queue ahead of the Zs drain so the mp ring never stalls ----
            TT_next = Gp_next = None
            if c + 1 < NCHUNK:
                TT_next = emit_tt(c + 1)
            if c == 0:
                for eb in range(DB):
                    emit_wvt_build(eb)

            nc.gpsimd.tensor_add(racc, racc2[:, 0, :], racc2[:, 1, :])

            # ---- r chain: (PE transpose, DVE reduce, recip) ----
            rr_cols = []
            for qb in range(QC // P):
                tp = mp.tile([P, 512], F32R, tag="mp", name="rtp")
                nc.tensor.transpose(tp[:, 0:P],
                                    racc[:, qb * P:(qb + 1) * P], identR)
                rcol = smal.tile([P, 1], F32, tag="rcol", name="rcol")
                nc.vector.reduce_sum(out=rcol, in_=tp[:, 0:P].bitcast(F32),
                                     axis=mybir.AxisListType.X)
                rr = smal.tile([P, 1], F32, tag="rr", name="rr")
                nc.vector.reciprocal(rr, rcol)
                rr_cols.append(rr)

            # ---- Z psum -> sbuf bf16, split across DVE + Act ----
            Zs = chk.tile([P, DB, QC], BF16, tag="zs", name="Zs")
            nc.vector.tensor_copy(Zs[:, :, 0:P], zp[:, :, 0:P])
            nc.scalar.activation(Zs[:, :, P:QC], zp[:, :, P:QC],
                                 mybir.ActivationFunctionType.Copy)

            if c + 1 < NCHUNK:
                Gp_next = emit_gp(TT_next)

            # ---- O = (Z^T.T @ Wv^T) * (1/r) + bv ----
            for qb in range(QC // P):
                ot = osb.tile([P, D], F32, tag="ot", name="ot")
                for ec in range(2):
                    op_ps = mp.tile([P, 512], F32, tag="mp", name="op_ps")
                    for db in range(DB):
                        nc.tensor.matmul(
                            op_ps,
                            Zs[:, db, qb * P:(qb + 1) * P],
                            WvT[:, db, ec * 512:(ec + 1) * 512],
                            start=(db == 0), stop=(db == DB - 1),
                        )
                    nc.vector.tensor_scalar_mul(
                        ot[:, ec * 512:(ec + 1) * 512], op_ps, rr_cols[qb])
                    nc.gpsimd.tensor_add(
                        ot[:, ec * 512:(ec + 1) * 512],
                        ot[:, ec * 512:(ec + 1) * 512],
                        bv_bc[:, ec * 512:(ec + 1) * 512])
                nc.scalar.dma_start(outp[q0 + qb * P:q0 + (qb + 1) * P, :], ot)
            TT, Gp = TT_next, Gp_next


def _get_nc():
    if "nc" not in _CACHE:
        _CACHE["nc"] = _build()
    return _CACHE["nc"]


def kernel(**inputs):
    inp = {k: np.ascontiguousarray(np.asarray(v, dtype=np.float32))
           for k, v in inputs.items()}
    nc = _get_nc()
    in_maps = [
        {
            "target": inp["target"][b],
            "non_target": inp["non_target"][b],
            "Wq": inp["Wq"], "Wk": inp["Wk"], "Wv": inp["Wv"],
            "bq": inp["bq"], "bv": inp["bv"],
        }
        for b in range(NCORES)
    ]
    res = run_bass_kernel_spmd(nc, in_maps, list(range(NCORES)))
    _CACHE["last_result"] = res
    out = np.stack([res.results[b]["out"] for b in range(NCORES)], axis=0)
    return out


# revision 18
# speedup vs baseline: 1.0296x; 1.0296x over previous
"""Fused attention kernel for TRN2, 8 NeuronCores, data-parallel over batch.

Problem: q = target @ Wq.T + bq ; k = non_target @ Wk.T + bk ;
         v = non_target @ Wv.T + bv ; out = softmax(q k^T) v
Shapes: target/non_target [8, 2048, 1024], W* [1024, 1024], b* [1024].

Math (per batch, one core):
  softmax(q k^T) is row-shift invariant, so kv-constant terms drop:
    S' = T M N^T + 1 w^T,  M = Wq^T Wk,  w = N (Wk^T bq)   (bk drops out)
  Work transposed: S'^T = N G',  G'[d',q] = sum_d M[d,d'] T^T[d,q] + y[d'],
  y = Wk^T bq.  P^T = exp(S'^T) unnormalized, r[q] = sum_kv P^T[kv,q],
    O[q,e] = ( sum_d Z^T[d,q] Wv^T[d,e] ) / r[q] + bv[e]
    Z^T[d,q] = sum_kv N[kv,d] P^T[kv,q]

Scores path (S') stays fp32r end-to-end.  The value path (P, N-for-Z, Z,
Wv^T) is bf16: rel tolerance is 2e-2 and bf16 rounding there only perturbs
the weighted average of v, not the softmax logits.  (The PE requires both
matmul operands f32/f32r or both 16-bit, so the paths never mix dtypes.)

Residency: N^T (f32r, 8MB), N (bf16, 4MB), M (f32r, 4MB), Wv^T (bf16, 2MB)
all live in SBUF, so steady-state HBM traffic is only T in / O out
(2MB/chunk).  N and Wv land once; N^T / bf16-N builds are pipelined into
chunk 0's kv loop, Wv^T builds into chunk 0's tail.  M is built with
eb-outer accumulation across all 8 PSUM banks so its matmuls trail the
per-half-block Wq/Wk DMAs instead of waiting for the full weight load.
"""

import numpy as np

import concourse.bass as bass
import concourse.mybir as mybir
import concourse.tile as tile
from concourse import bacc
from concourse.bass_utils import run_bass_kernel_spmd
from concourse.masks import make_identity

F32 = mybir.dt.float32
F32R = mybir.dt.float32r
BF16 = mybir.dt.bfloat16

B, SQ, SKV, D = 8, 2048, 2048, 1024
P = 128
QC = 256                 # q-chunk size
NCHUNK = SQ // QC        # 8
DB = D // P              # 8 d-blocks
KVB = SKV // P           # 16 kv-blocks
NPAIR = KVB // 2         # 8 kv-pairs per chunk
NCORES = 8

_CACHE = {}


def _build():
    nc = bacc.Bacc()
    tgt = nc.declare_dram_parameter("target", [SQ, D], F32R, isOutput=False)
    ntg = nc.declare_dram_parameter("non_target", [SKV, D], F32R, isOutput=False)
    wqp = nc.declare_dram_parameter("Wq", [D, D], F32R, isOutput=False)
    wkp = nc.declare_dram_parameter("Wk", [D, D], F32R, isOutput=False)
    wvp = nc.declare_dram_parameter("Wv", [D, D], F32R, isOutput=False)
    bqp = nc.declare_dram_parameter("bq", [D], F32, isOutput=False)
    bvp = nc.declare_dram_parameter("bv", [D], F32, isOutput=False)
    outp = nc.declare_dram_parameter("out", [SQ, D], F32, isOutput=True)
    with tile.TileContext(nc) as tc:
        _emit(nc, tc, tgt, ntg, wqp, wkp, wvp, bqp, bvp, outp)
    nc.compile()
    return nc


def _emit(nc, tc, tgt, ntg, wqp, wkp, wvp, bqp, bvp, outp):
    import contextlib
    ctx = contextlib.ExitStack()
    with ctx:
        # ---- small residents ----
        R = ctx.enter_context(tc.tile_pool(name="resident", bufs=1))
        identF = R.tile([P, P], F32)
        make_identity(nc, identF)
        identR = R.tile([P, P], F32R)
        nc.vector.tensor_copy(identR, identF)
        Mt = R.tile([P, DB, D], F32R)       # M: [d in block, d-block, d']
        y_col = R.tile([P, DB], F32)        # y: [d' in block, d'-block]
        w_col = R.tile([P, KVB], F32)       # w = N y: [kv in block, kv-block]
        bq_col = R.tile([P, DB], F32)       # bq: [e in block, e-block]
        bv_bc = R.tile([P, D], BF16)        # bv broadcast to 128 partitions
        nc.sync.dma_start(bq_col, bqp[:].rearrange("(b p) -> p b", p=P))
        bv_bcast_ap = bass.AP(
            tensor=bvp[:].tensor, offset=0,
            ap=[[0, P], [1, D]],
        )
        nc.gpsimd.dma_start(out=bv_bc, in_=bv_bcast_ap)

        # ---- N^T resident (f32r, scores path) ----
        Rnt = ctx.enter_context(tc.tile_pool(name="rnt", bufs=1))
        NT = Rnt.tile([P, DB, SKV], F32R)   # N^T: [d' in block, d'-block, kv]
        # N resident in bf16 for the Z matmuls (value path); the last 4
        # kv-tiles live in the post-wtmp region (their conversions run in
        # chunk 0, after the W tiles are dead)
        NZ_MAIN = 12
        Rnz = ctx.enter_context(tc.tile_pool(name="rnz", bufs=1))
        Nz = Rnz.tile([P, NZ_MAIN, D], BF16)  # N: [kv in block, kv-block, d]
        # N / T load rings get their own SBUF so their DMAs never wait on
        # the W-region reads (pools allocated after wtmp alias W's space)
        nld = ctx.enter_context(tc.tile_pool(name="nld", bufs=2))
        tld = ctx.enter_context(tc.tile_pool(name="tld", bufs=2))

        ring_tiles = {}
        tl_tiles = {}

        def emit_nload(j):
            nl = nld.tile([P, D], F32R, tag="nld", name="nl")
            eng = nc.sync if j % 2 == 0 else nc.scalar
            eng.dma_start(nl, ntg[j * P:(j + 1) * P, :])
            ring_tiles[("n", j)] = nl

        def emit_tload(c):
            q0 = c * QC
            for qb in range(QC // P):
                tl = tld.tile([P, D], F32R, tag="tload", name="tl")
                nc.sync.dma_start(tl, tgt[q0 + qb * P:q0 + (qb + 1) * P, :])
                tl_tiles[(c, qb)] = tl

        # ---- M = Wq^T Wk, arrival-ordered groups over per-half-block DMAs;
        # NT builds interleave into the later passes so the N stream and the
        # PE both stay busy while M finishes ----
        with tc.tile_pool(name="wtmp", bufs=1) as W, \
             tc.tile_pool(name="pp6", bufs=1, space="PSUM") as pp6, \
             tc.tile_pool(name="ppnt", bufs=2, space="PSUM") as ppnt:
            wq_eb = [W.tile([P, D], F32R, name=f"wq{eb}") for eb in range(DB)]
            wk_eb = [W.tile([P, D], F32R, name=f"wk{eb}") for eb in range(DB)]
            for eb in range(DB):
                nc.sync.dma_start(wq_eb[eb][:, 0:512],
                                  wqp[eb * P:(eb + 1) * P, 0:512])
                nc.scalar.dma_start(wk_eb[eb][:, 0:512],
                                    wkp[eb * P:(eb + 1) * P, 0:512])
                nc.sync.dma_start(wq_eb[eb][:, 512:D],
                                  wqp[eb * P:(eb + 1) * P, 512:D])
                nc.scalar.dma_start(wk_eb[eb][:, 512:D],
                                    wkp[eb * P:(eb + 1) * P, 512:D])
            # N and T queued right behind W: T slotted after N5 so chunk 0's
            # T^T build is ready well before the Gp0 matmuls need it
            for j in range(6):
                emit_nload(j)
            emit_tload(0)
            for j in range(6, KVB):
                emit_nload(j)

            def emit_nt_build(j, pool=None):
                """N^T tile j (PE transposes) + bf16 N copy + w column."""
                pool = pool or ppnt
                nl = ring_tiles.pop(("n", j))
                for g in range(2):
                    tp = pool.tile([P, 512], F32R, tag="mp", name="ntp")
                    tpv = tp.rearrange("p (k c) -> p k c", k=4)
                    for k in range(4):
                        db = 4 * g + k
                        nc.tensor.transpose(tpv[:, k, :],
                                            nl[:, db * P:(db + 1) * P], identR)
                    dst = NT[:, 4 * g:4 * g + 4, j * P:(j + 1) * P]
                    if g == 0:
                        nc.vector.tensor_copy(dst, tpv)
                    else:
                        nc.scalar.activation(dst, tpv,
                                             mybir.ActivationFunctionType.Copy)
                nzt = Nz[:, j, :] if j < NZ_MAIN else Nz_tail[:, j - NZ_MAIN, :]
                if j % 2 == 0:
                    nc.gpsimd.tensor_copy(nzt, nl)
                else:
                    nc.scalar.activation(nzt, nl,
                                         mybir.ActivationFunctionType.Copy)
                # w[kv] = sum_d' N[kv,d'] y[d'] — the per-kv score shift from
                # bq, fused into exp's bias (no per-chunk Gp bias adds)
                wp = pool.tile([P, 512], F32, tag="mp", name="wp")
                for ob in range(DB):
                    nc.tensor.matmul(
                        wp[:, 0:1],
                        NT[:, ob, j * P:(j + 1) * P].bitcast(F32),
                        y_col[:, ob:ob + 1],
                        start=(ob == 0), stop=(ob == DB - 1),
                    )
                nc.vector.tensor_copy(w_col[:, j:j + 1], wp[:, 0:1])

            # group order matches DMA arrival: db<=3/ch0 need only the first
            # halves; db>=4 and ch1 groups need the later halves
            PASS_A = [(0, 0), (1, 0), (2, 0), (3, 0), (0, 1), (1, 1)]
            PASS_B1 = [(2, 1), (3, 1), (4, 0), (5, 0), (6, 0)]
            PASS_B2 = [(7, 0), (4, 1), (5, 1), (6, 1), (7, 1)]

            def m_pass(groups):
                mps = [pp6.tile([P, 512], F32, tag=f"m{g}", name=f"mps{g}")
                       for g in range(len(groups))]
                for eb in range(DB):
                    for g, (db, ch) in enumerate(groups):
                        nc.tensor.matmul(
                            mps[g],
                            wq_eb[eb][:, db * P:(db + 1) * P],
                            wk_eb[eb][:, ch * 512:(ch + 1) * 512],
                            start=(eb == 0), stop=(eb == DB - 1),
                        )
                    yield eb
                for g, (db, ch) in enumerate(groups):
                    if g % 2 == 0:
                        nc.vector.tensor_copy(Mt[:, db, ch * 512:(ch + 1) * 512],
                                              mps[g])
                    else:
                        nc.scalar.activation(Mt[:, db, ch * 512:(ch + 1) * 512],
                                             mps[g],
                                             mybir.ActivationFunctionType.Copy)

            for _ in m_pass(PASS_A):
                pass
            # B passes: ~1.5 NT builds per eb iteration keeps pace with the
            # 1.46us/tile N DMA stream
            nt_next = 0
            NT_SCHED = {1: 2, 2: 1, 3: 2, 4: 1, 5: 2, 6: 1, 7: 2}
            for eb in m_pass(PASS_B1):
                for _ in range(NT_SCHED.get(eb, 0)):
                    if nt_next < NZ_MAIN:
                        emit_nt_build(nt_next)
                        nt_next += 1
            for eb in m_pass(PASS_B2):
                for _ in range(NT_SCHED.get(eb, 0)):
                    if nt_next < NZ_MAIN:
                        emit_nt_build(nt_next)
                        nt_next += 1
            while nt_next < NZ_MAIN:
                emit_nt_build(nt_next)
                nt_next += 1
            # y = Wk^T bq (tiny matmuls)
            for ob in range(DB):
                yp = pp6.tile([P, 512], F32, tag="m0", name="yp")
                for eb in range(DB):
                    nc.tensor.matmul(
                        yp[:, 0:1],
                        wk_eb[eb][:, ob * P:(ob + 1) * P].bitcast(F32),
                        bq_col[:, eb:eb + 1],
                        start=(eb == 0), stop=(eb == DB - 1),
                    )
                nc.vector.tensor_copy(y_col[:, ob:ob + 1], yp[:, 0:1])

        # ---- pools that live from here on (fit in space freed by wtmp) ----
        Rnzt = ctx.enter_context(tc.tile_pool(name="rnzt", bufs=1))
        Nz_tail = Rnzt.tile([P, KVB - NZ_MAIN, D], BF16)
        Rwv = ctx.enter_context(tc.tile_pool(name="rwv", bufs=1))
        WvT = Rwv.tile([P, DB, D], BF16)    # Wv^T: [d in block, d-block, e]

        # ---- main-loop pools ----
        sp = ctx.enter_context(tc.tile_pool(name="sp", bufs=2, space="PSUM"))
        zpp = ctx.enter_context(tc.tile_pool(name="zp", bufs=1, space="PSUM"))
        mp = ctx.enter_context(tc.tile_pool(name="mp", bufs=2, space="PSUM"))
        chk = ctx.enter_context(tc.tile_pool(name="chk", bufs=1))
        ptp = ctx.enter_context(tc.tile_pool(name="ptp", bufs=3))
        osb = ctx.enter_context(tc.tile_pool(name="osb", bufs=2))
        smal = ctx.enter_context(tc.tile_pool(name="smal", bufs=4))

        # Wv rides the nld ring (on the scalar queue only, so the sync queue
        # stays clear for T prefetches); its WvT builds pipeline into chunk
        # 0's kv loop
        def emit_wvload(eb):
            wv = nld.tile([P, D], F32R, tag="nld", name="wv")
            nc.scalar.dma_start(wv, wvp[eb * P:(eb + 1) * P, :])
            ring_tiles[("wv", eb)] = wv

        for eb in range(DB):
            emit_wvload(eb)

        def emit_wvt_build(eb):
            """Wv^T columns for e-block eb (f32r transposes; the PSUM->SBUF
            drain copies convert to bf16)."""
            wv = ring_tiles.pop(("wv", eb))
            for g in range(2):
                tp = mp.tile([P, 512], F32R, tag="mp", name="tpv")
                tpv = tp.rearrange("p (k c) -> p k c", k=4)
                for k in range(4):
                    db = 4 * g + k
                    nc.tensor.transpose(tpv[:, k, :], wv[:, db * P:(db + 1) * P],
                                        identR)
                dst = WvT[:, 4 * g:4 * g + 4, eb * P:(eb + 1) * P]
                if g == 0:
                    nc.vector.tensor_copy(dst, tpv)
                else:
                    nc.scalar.activation(dst, tpv,
                                         mybir.ActivationFunctionType.Copy)

        def emit_tt(c):
            """T^T for chunk c from prefetched tl tiles."""
            TT = chk.tile([P, DB, QC], F32R, tag="tt", name="TT")
            for qb in range(QC // P):
                tl = tl_tiles.pop((c, qb))
                for g in range(2):
                    tp = mp.tile([P, 512], F32R, tag="mp", name="tp")
                    tpv = tp.rearrange("p (k c) -> p k c", k=4)
                    for k in range(4):
                        db = 4 * g + k
                        nc.tensor.transpose(tpv[:, k, :],
                                            tl[:, db * P:(db + 1) * P], identR)
                    dst = TT[:, 4 * g:4 * g + 4, qb * P:(qb + 1) * P]
                    if g == 0:
                        nc.vector.tensor_copy(dst, tpv)
                    else:
                        nc.scalar.activation(dst, tpv,
                                             mybir.ActivationFunctionType.Copy)
            return TT

        def emit_gp(TT):
            Gp = chk.tile([P, DB, QC], F32R, tag="gp", name="Gp")
            for ob in range(DB):
                gp_ps = mp.tile([P, 512], F32, tag="mp", name="gp_ps")
                for db in range(DB):
                    nc.tensor.matmul(
                        gp_ps[:, 0:QC],
                        Mt[:, db, ob * P:(ob + 1) * P],
                        TT[:, db, :],
                        start=(db == 0), stop=(db == DB - 1),
                    )
                if ob % 2 == 0:
                    nc.vector.tensor_copy(Gp[:, ob, :], gp_ps[:, 0:QC])
                else:
                    nc.scalar.activation(Gp[:, ob, :], gp_ps[:, 0:QC],
                                         mybir.ActivationFunctionType.Copy)
            return Gp

        TT = emit_tt(0)
        Gp = emit_gp(TT)

        for c in range(NCHUNK):
            q0 = c * QC
            if c + 1 < NCHUNK:
                emit_tload(c + 1)   # prefetch next chunk's T now
            # ---- kv loop: S' -> exp -> racc ; Z pipelined one pair behind ----
            zp = zpp.tile([P, DB, QC], F32, name="zp")
            racc2 = chk.tile([P, 2, QC], F32, tag="racc2", name="racc2")
            racc = chk.tile([P, QC], F32R, tag="racc", name="racc")
            pts = {}
            for jj in range(NPAIR + 1):
                if jj < NPAIR:
                    spt = sp.tile([P, 2, QC], F32, name="spt")
                    for h in range(2):
                        j = 2 * jj + h
                        for ob in range(DB):
                            nc.tensor.matmul(
                                spt[:, h, :],
                                NT[:, ob, j * P:(j + 1) * P],
                                Gp[:, ob, :],
                                start=(ob == 0), stop=(ob == DB - 1),
                            )
                    pt = ptp.tile([P, 2, QC], BF16, tag="pt", name="pt")
                    for h in range(2):
                        j = 2 * jj + h
                        nc.scalar.activation(pt[:, h, :], spt[:, h, :],
                                             mybir.ActivationFunctionType.Exp,
                                             bias=w_col[:, j:j + 1])
                    pts[jj] = pt
                    if jj == 0:
                        nc.gpsimd.tensor_copy(racc2, pt)
                    else:
                        nc.gpsimd.tensor_add(racc2, racc2, pt)
                    if c == 0:
                        # pipeline the remaining one-time builds into chunk 0
                        if jj < 2:
                            emit_nt_build(NZ_MAIN + 2 * jj, pool=mp)
                            emit_nt_build(NZ_MAIN + 2 * jj + 1, pool=mp)
                        else:
                            emit_wvt_build(jj - 2)
                if jj > 0:
                    zjj = jj - 1
                    pt = pts.pop(zjj)
                    for h in range(2):
                        j = 2 * zjj + h
                        for db in range(DB):
                            # start only on the even db of each 2KB PSUM zero
                            # region: its start marks the whole region, and
                            # the odd db's first write lands as overwrite
                            # (a second start would re-mark the even db's
                            # fresh j=0 data as pending-zero and lose it)
                            nzsrc = (Nz[:, j, db * P:(db + 1) * P]
                                     if j < NZ_MAIN else
                                     Nz_tail[:, j - NZ_MAIN, db * P:(db + 1) * P])
                            nc.tensor.matmul(
                                zp[:, db, :],
                                nzsrc,
                                pt[:, h, :],
                                start=(j == 0 and db % 2 == 0),
                                stop=(j == KVB - 1),
                                skip_group_check=True,
                            )

            if c == 0:
                emit_wvt_build(6)
                emit_wvt_build(7)

            # ---- Z psum -> sbuf bf16: 4-way split across DVE/Act so each
            # piece lands fast and the O matmuls / next chunk unblock early ----
            Zs = chk.tile([P, DB, QC], BF16, tag="zs", name="Zs")
            nc.vector.tensor_copy(Zs[:, 0:4, 0:P], zp[:, 0:4, 0:P])
            nc.scalar.activation(Zs[:, 4:8, 0:P], zp[:, 4:8, 0:P],
                                 mybir.ActivationFunctionType.Copy)
            nc.vector.tensor_copy(Zs[:, 0:4, P:QC], zp[:, 0:4, P:QC])
            nc.scalar.activation(Zs[:, 4:8, P:QC], zp[:, 4:8, P:QC],
                                 mybir.ActivationFunctionType.Copy)

            TT_next = Gp_next = None
            if c + 1 < NCHUNK:
                TT_next = emit_tt(c + 1)

            nc.gpsimd.tensor_add(racc, racc2[:, 0, :], racc2[:, 1, :])

            # ---- r chain: (PE transpose, DVE reduce, recip) ----
            rr_cols = []
            for qb in range(QC // P):
                tp = mp.tile([P, 512], F32R, tag="mp", name="rtp")
                nc.tensor.transpose(tp[:, 0:P],
                                    racc[:, qb * P:(qb + 1) * P], identR)
                rcol = smal.tile([P, 1], F32, tag="rcol", name="rcol")
                nc.vector.reduce_sum(out=rcol, in_=tp[:, 0:P].bitcast(F32),
                                     axis=mybir.AxisListType.X)
                rr = smal.tile([P, 1], F32, tag="rr", name="rr")
                nc.vector.reciprocal(rr, rcol)
                rr_cols.append(rr)

            if c + 1 < NCHUNK:
                Gp_next = emit_gp(TT_next)

            # ---- O = (Z^T.T @ Wv^T) * (1/r) + bv ----
            for qb in range(QC // P):
                ot = osb.tile([P, D], F32, tag="ot", name="ot")
                for ec in range(2):
                    op_ps = mp.tile([P, 512], F32, tag="mp", name="op_ps")
                    for db in range(DB):
                        nc.tensor.matmul(
                            op_ps,
                            Zs[:, db, qb * P:(qb + 1) * P],
                            WvT[:, db, ec * 512:(ec + 1) * 512],
                            start=(db == 0), stop=(db == DB - 1),
                        )
                    nc.vector.tensor_scalar_mul(
                        ot[:, ec * 512:(ec + 1) * 512], op_ps, rr_cols[qb])
                    nc.gpsimd.tensor_add(
                        ot[:, ec * 512:(ec + 1) * 512],
                        ot[:, ec * 512:(ec + 1) * 512],
                        bv_bc[:, ec * 512:(ec + 1) * 512])
                    nc.scalar.dma_start(
                        outp[q0 + qb * P:q0 + (qb + 1) * P,
                             ec * 512:(ec + 1) * 512],
                        ot[:, ec * 512:(ec + 1) * 512])
            TT, Gp = TT_next, Gp_next


def _get_nc():
    if "nc" not in _CACHE:
        _CACHE["nc"] = _build()
    return _CACHE["nc"]


def kernel(**inputs):
    inp = {k: np.ascontiguousarray(np.asarray(v, dtype=np.float32))
           for k, v in inputs.items()}
    nc = _get_nc()
    in_maps = [
        {
            "target": inp["target"][b],
            "non_target": inp["non_target"][b],
            "Wq": inp["Wq"], "Wk": inp["Wk"], "Wv": inp["Wv"],
            "bq": inp["bq"], "bv": inp["bv"],
        }
        for b in range(NCORES)
    ]
    res = run_bass_kernel_spmd(nc, in_maps, list(range(NCORES)))
    _CACHE["last_result"] = res
    out = np.stack([res.results[b]["out"] for b in range(NCORES)], axis=0)
    return out


# revision 19
# speedup vs baseline: 1.0315x; 1.0018x over previous
"""Fused attention kernel for TRN2, 8 NeuronCores, data-parallel over batch.

Problem: q = target @ Wq.T + bq ; k = non_target @ Wk.T + bk ;
         v = non_target @ Wv.T + bv ; out = softmax(q k^T) v
Shapes: target/non_target [8, 2048, 1024], W* [1024, 1024], b* [1024].

Math (per batch, one core):
  softmax(q k^T) is row-shift invariant, so kv-constant terms drop:
    S' = T M N^T + 1 w^T,  M = Wq^T Wk,  w = N (Wk^T bq)   (bk drops out)
  Work transposed: S'^T = N G',  G'[d',q] = sum_d M[d,d'] T^T[d,q] + y[d'],
  y = Wk^T bq.  P^T = exp(S'^T) unnormalized, r[q] = sum_kv P^T[kv,q],
    O[q,e] = ( sum_d Z^T[d,q] Wv^T[d,e] ) / r[q] + bv[e]
    Z^T[d,q] = sum_kv N[kv,d] P^T[kv,q]

Scores path (S') stays fp32r end-to-end.  The value path (P, N-for-Z, Z,
Wv^T) is bf16: rel tolerance is 2e-2 and bf16 rounding there only perturbs
the weighted average of v, not the softmax logits.  (The PE requires both
matmul operands f32/f32r or both 16-bit, so the paths never mix dtypes.)

Residency: N^T (f32r, 8MB), N (bf16, 4MB), M (f32r, 4MB), Wv^T (bf16, 2MB)
all live in SBUF, so steady-state HBM traffic is only T in / O out
(2MB/chunk).  N and Wv land once; N^T / bf16-N builds are pipelined into
chunk 0's kv loop, Wv^T builds into chunk 0's tail.  M is built with
eb-outer accumulation across all 8 PSUM banks so its matmuls trail the
per-half-block Wq/Wk DMAs instead of waiting for the full weight load.
"""

import numpy as np

import concourse.bass as bass
import concourse.mybir as mybir
import concourse.tile as tile
from concourse import bacc
from concourse.bass_utils import run_bass_kernel_spmd
from concourse.masks import make_identity

F32 = mybir.dt.float32
F32R = mybir.dt.float32r
BF16 = mybir.dt.bfloat16

B, SQ, SKV, D = 8, 2048, 2048, 1024
P = 128
QC = 256                 # q-chunk size
NCHUNK = SQ // QC        # 8
DB = D // P              # 8 d-blocks
KVB = SKV // P           # 16 kv-blocks
NPAIR = KVB // 2         # 8 kv-pairs per chunk
NCORES = 8

_CACHE = {}


def _build():
    nc = bacc.Bacc()
    tgt = nc.declare_dram_parameter("target", [SQ, D], F32R, isOutput=False)
    ntg = nc.declare_dram_parameter("non_target", [SKV, D], F32R, isOutput=False)
    wqp = nc.declare_dram_parameter("Wq", [D, D], F32R, isOutput=False)
    wkp = nc.declare_dram_parameter("Wk", [D, D], F32R, isOutput=False)
    wvp = nc.declare_dram_parameter("Wv", [D, D], F32R, isOutput=False)
    bqp = nc.declare_dram_parameter("bq", [D], F32, isOutput=False)
    bvp = nc.declare_dram_parameter("bv", [D], F32, isOutput=False)
    outp = nc.declare_dram_parameter("out", [SQ, D], F32, isOutput=True)
    with tile.TileContext(nc) as tc:
        _emit(nc, tc, tgt, ntg, wqp, wkp, wvp, bqp, bvp, outp)
    nc.compile()
    return nc


def _emit(nc, tc, tgt, ntg, wqp, wkp, wvp, bqp, bvp, outp):
    import contextlib
    ctx = contextlib.ExitStack()
    with ctx:
        # ---- small residents ----
        R = ctx.enter_context(tc.tile_pool(name="resident", bufs=1))
        identF = R.tile([P, P], F32)
        make_identity(nc, identF)
        identR = R.tile([P, P], F32R)
        nc.vector.tensor_copy(identR, identF)
        Mt = R.tile([P, DB, D], F32R)       # M: [d in block, d-block, d']
        y_col = R.tile([P, DB], F32)        # y: [d' in block, d'-block]
        w_col = R.tile([P, KVB], F32)       # w = N y: [kv in block, kv-block]
        bq_col = R.tile([P, DB], F32)       # bq: [e in block, e-block]
        bv_bc = R.tile([P, D], BF16)        # bv broadcast to 128 partitions
        nc.sync.dma_start(bq_col, bqp[:].rearrange("(b p) -> p b", p=P))
        bv_bcast_ap = bass.AP(
            tensor=bvp[:].tensor, offset=0,
            ap=[[0, P], [1, D]],
        )
        nc.gpsimd.dma_start(out=bv_bc, in_=bv_bcast_ap)

        # ---- N^T resident (f32r, scores path) ----
        Rnt = ctx.enter_context(tc.tile_pool(name="rnt", bufs=1))
        NT = Rnt.tile([P, DB, SKV], F32R)   # N^T: [d' in block, d'-block, kv]
        # N resident in bf16 for the Z matmuls (value path); the last 4
        # kv-tiles live in the post-wtmp region (their conversions run in
        # chunk 0, after the W tiles are dead)
        NZ_MAIN = 12
        Rnz = ctx.enter_context(tc.tile_pool(name="rnz", bufs=1))
        Nz = Rnz.tile([P, NZ_MAIN, D], BF16)  # N: [kv in block, kv-block, d]
        # N / T load rings get their own SBUF so their DMAs never wait on
        # the W-region reads (pools allocated after wtmp alias W's space)
        nld = ctx.enter_context(tc.tile_pool(name="nld", bufs=2))
        tld = ctx.enter_context(tc.tile_pool(name="tld", bufs=2))

        ring_tiles = {}
        tl_tiles = {}

        def emit_nload(j):
            nl = nld.tile([P, D], F32R, tag="nld", name="nl")
            eng = nc.sync if j % 2 == 0 else nc.scalar
            eng.dma_start(nl, ntg[j * P:(j + 1) * P, :])
            ring_tiles[("n", j)] = nl

        def emit_tload(c):
            q0 = c * QC
            for qb in range(QC // P):
                tl = tld.tile([P, D], F32R, tag="tload", name="tl")
                nc.sync.dma_start(tl, tgt[q0 + qb * P:q0 + (qb + 1) * P, :])
                tl_tiles[(c, qb)] = tl

        # ---- M = Wq^T Wk, arrival-ordered groups over per-half-block DMAs;
        # NT builds interleave into the later passes so the N stream and the
        # PE both stay busy while M finishes ----
        with tc.tile_pool(name="wtmp", bufs=1) as W, \
             tc.tile_pool(name="pp6", bufs=1, space="PSUM") as pp6, \
             tc.tile_pool(name="ppnt", bufs=2, space="PSUM") as ppnt:
            wq_eb = [W.tile([P, D], F32R, name=f"wq{eb}") for eb in range(DB)]
            wk_eb = [W.tile([P, D], F32R, name=f"wk{eb}") for eb in range(DB)]
            for eb in range(DB):
                nc.sync.dma_start(wq_eb[eb][:, 0:512],
                                  wqp[eb * P:(eb + 1) * P, 0:512])
                nc.scalar.dma_start(wk_eb[eb][:, 0:512],
                                    wkp[eb * P:(eb + 1) * P, 0:512])
                nc.sync.dma_start(wq_eb[eb][:, 512:D],
                                  wqp[eb * P:(eb + 1) * P, 512:D])
                nc.scalar.dma_start(wk_eb[eb][:, 512:D],
                                    wkp[eb * P:(eb + 1) * P, 512:D])
            # N and T queued right behind W: T slotted after N5 so chunk 0's
            # T^T build is ready well before the Gp0 matmuls need it
            for j in range(6):
                emit_nload(j)
            emit_tload(0)
            for j in range(6, KVB):
                emit_nload(j)

            def emit_nt_build(j, pool=None):
                """N^T tile j (PE transposes) + bf16 N copy + w column."""
                pool = pool or ppnt
                nl = ring_tiles.pop(("n", j))
                for g in range(2):
                    tp = pool.tile([P, 512], F32R, tag="mp", name="ntp")
                    tpv = tp.rearrange("p (k c) -> p k c", k=4)
                    for k in range(4):
                        db = 4 * g + k
                        nc.tensor.transpose(tpv[:, k, :],
                                            nl[:, db * P:(db + 1) * P], identR)
                    dst = NT[:, 4 * g:4 * g + 4, j * P:(j + 1) * P]
                    if g == 0:
                        nc.vector.tensor_copy(dst, tpv)
                    else:
                        nc.scalar.activation(dst, tpv,
                                             mybir.ActivationFunctionType.Copy)
                nzt = Nz[:, j, :] if j < NZ_MAIN else Nz_tail[:, j - NZ_MAIN, :]
                if j % 2 == 0:
                    nc.gpsimd.tensor_copy(nzt, nl)
                else:
                    nc.scalar.activation(nzt, nl,
                                         mybir.ActivationFunctionType.Copy)
                # w[kv] = sum_d' N[kv,d'] y[d'] — the per-kv score shift from
                # bq, fused into exp's bias (no per-chunk Gp bias adds)
                wp = pool.tile([P, 512], F32, tag="mp", name="wp")
                for ob in range(DB):
                    nc.tensor.matmul(
                        wp[:, 0:1],
                        NT[:, ob, j * P:(j + 1) * P].bitcast(F32),
                        y_col[:, ob:ob + 1],
                        start=(ob == 0), stop=(ob == DB - 1),
                    )
                nc.vector.tensor_copy(w_col[:, j:j + 1], wp[:, 0:1])

            # group order matches DMA arrival: db<=3/ch0 need only the first
            # halves; db>=4 and ch1 groups need the later halves
            PASS_A = [(0, 0), (1, 0), (2, 0), (3, 0), (0, 1), (1, 1)]
            PASS_B1 = [(2, 1), (3, 1), (4, 0), (5, 0), (6, 0)]
            PASS_B2 = [(7, 0), (4, 1), (5, 1), (6, 1), (7, 1)]

            def m_pass(groups):
                mps = [pp6.tile([P, 512], F32, tag=f"m{g}", name=f"mps{g}")
                       for g in range(len(groups))]
                for eb in range(DB):
                    for g, (db, ch) in enumerate(groups):
                        nc.tensor.matmul(
                            mps[g],
                            wq_eb[eb][:, db * P:(db + 1) * P],
                            wk_eb[eb][:, ch * 512:(ch + 1) * 512],
                            start=(eb == 0), stop=(eb == DB - 1),
                        )
                    yield eb
                for g, (db, ch) in enumerate(groups):
                    if g % 2 == 0:
                        nc.vector.tensor_copy(Mt[:, db, ch * 512:(ch + 1) * 512],
                                              mps[g])
                    else:
                        nc.scalar.activation(Mt[:, db, ch * 512:(ch + 1) * 512],
                                             mps[g],
                                             mybir.ActivationFunctionType.Copy)

            for _ in m_pass(PASS_A):
                pass
            # y = Wk^T bq (tiny matmuls) — must precede the NT builds, whose
            # w-column matmuls read y_col
            for ob in range(DB):
                yp = ppnt.tile([P, 512], F32, tag="mp", name="yp")
                for eb in range(DB):
                    nc.tensor.matmul(
                        yp[:, 0:1],
                        wk_eb[eb][:, ob * P:(ob + 1) * P].bitcast(F32),
                        bq_col[:, eb:eb + 1],
                        start=(eb == 0), stop=(eb == DB - 1),
                    )
                nc.vector.tensor_copy(y_col[:, ob:ob + 1], yp[:, 0:1])
            # B passes: ~1.5 NT builds per eb iteration keeps pace with the
            # 1.46us/tile N DMA stream
            nt_next = 0
            NT_SCHED = {1: 2, 2: 1, 3: 2, 4: 1, 5: 2, 6: 1, 7: 2}
            for eb in m_pass(PASS_B1):
                for _ in range(NT_SCHED.get(eb, 0)):
                    if nt_next < NZ_MAIN:
                        emit_nt_build(nt_next)
                        nt_next += 1
            for eb in m_pass(PASS_B2):
                for _ in range(NT_SCHED.get(eb, 0)):
                    if nt_next < NZ_MAIN:
                        emit_nt_build(nt_next)
                        nt_next += 1
            while nt_next < NZ_MAIN:
                emit_nt_build(nt_next)
                nt_next += 1

        # ---- pools that live from here on (fit in space freed by wtmp) ----
        Rnzt = ctx.enter_context(tc.tile_pool(name="rnzt", bufs=1))
        Nz_tail = Rnzt.tile([P, KVB - NZ_MAIN, D], BF16)
        Rwv = ctx.enter_context(tc.tile_pool(name="rwv", bufs=1))
        WvT = Rwv.tile([P, DB, D], BF16)    # Wv^T: [d in block, d-block, e]

        # ---- main-loop pools ----
        sp = ctx.enter_context(tc.tile_pool(name="sp", bufs=2, space="PSUM"))
        zpp = ctx.enter_context(tc.tile_pool(name="zp", bufs=1, space="PSUM"))
        mp = ctx.enter_context(tc.tile_pool(name="mp", bufs=2, space="PSUM"))
        chk = ctx.enter_context(tc.tile_pool(name="chk", bufs=1))
        ptp = ctx.enter_context(tc.tile_pool(name="ptp", bufs=3))
        osb = ctx.enter_context(tc.tile_pool(name="osb", bufs=2))
        smal = ctx.enter_context(tc.tile_pool(name="smal", bufs=4))

        # Wv rides the nld ring (on the scalar queue only, so the sync queue
        # stays clear for T prefetches); its WvT builds pipeline into chunk
        # 0's kv loop
        def emit_wvload(eb):
            wv = nld.tile([P, D], F32R, tag="nld", name="wv")
            nc.scalar.dma_start(wv, wvp[eb * P:(eb + 1) * P, :])
            ring_tiles[("wv", eb)] = wv

        for eb in range(DB):
            emit_wvload(eb)

        def emit_wvt_build(eb):
            """Wv^T columns for e-block eb (f32r transposes; the PSUM->SBUF
            drain copies convert to bf16)."""
            wv = ring_tiles.pop(("wv", eb))
            for g in range(2):
                tp = mp.tile([P, 512], F32R, tag="mp", name="tpv")
                tpv = tp.rearrange("p (k c) -> p k c", k=4)
                for k in range(4):
                    db = 4 * g + k
                    nc.tensor.transpose(tpv[:, k, :], wv[:, db * P:(db + 1) * P],
                                        identR)
                dst = WvT[:, 4 * g:4 * g + 4, eb * P:(eb + 1) * P]
                if g == 0:
                    nc.vector.tensor_copy(dst, tpv)
                else:
                    nc.scalar.activation(dst, tpv,
                                         mybir.ActivationFunctionType.Copy)

        def emit_tt(c):
            """T^T for chunk c from prefetched tl tiles."""
            TT = chk.tile([P, DB, QC], F32R, tag="tt", name="TT")
            for qb in range(QC // P):
                tl = tl_tiles.pop((c, qb))
                for g in range(2):
                    tp = mp.tile([P, 512], F32R, tag="mp", name="tp")
                    tpv = tp.rearrange("p (k c) -> p k c", k=4)
                    for k in range(4):
                        db = 4 * g + k
                        nc.tensor.transpose(tpv[:, k, :],
                                            tl[:, db * P:(db + 1) * P], identR)
                    dst = TT[:, 4 * g:4 * g + 4, qb * P:(qb + 1) * P]
                    if g == 0:
                        nc.vector.tensor_copy(dst, tpv)
                    else:
                        nc.scalar.activation(dst, tpv,
                                             mybir.ActivationFunctionType.Copy)
            return TT

        def emit_gp(TT):
            Gp = chk.tile([P, DB, QC], F32R, tag="gp", name="Gp")
            for ob in range(DB):
                gp_ps = mp.tile([P, 512], F32, tag="mp", name="gp_ps")
                for db in range(DB):
                    nc.tensor.matmul(
                        gp_ps[:, 0:QC],
                        Mt[:, db, ob * P:(ob + 1) * P],
                        TT[:, db, :],
                        start=(db == 0), stop=(db == DB - 1),
                    )
                if ob % 2 == 0:
                    nc.vector.tensor_copy(Gp[:, ob, :], gp_ps[:, 0:QC])
                else:
                    nc.scalar.activation(Gp[:, ob, :], gp_ps[:, 0:QC],
                                         mybir.ActivationFunctionType.Copy)
            return Gp

        TT = emit_tt(0)
        Gp = emit_gp(TT)

        for c in range(NCHUNK):
            q0 = c * QC
            if c + 1 < NCHUNK:
                emit_tload(c + 1)   # prefetch next chunk's T now
            # ---- kv loop: S' -> exp -> racc ; Z pipelined one pair behind ----
            zp = zpp.tile([P, DB, QC], F32, name="zp")
            racc2 = chk.tile([P, 2, QC], F32, tag="racc2", name="racc2")
            racc = chk.tile([P, QC], F32R, tag="racc", name="racc")
            pts = {}
            for jj in range(NPAIR + 1):
                if jj < NPAIR:
                    spt = sp.tile([P, 2, QC], F32, name="spt")
                    for h in range(2):
                        j = 2 * jj + h
                        for ob in range(DB):
                            nc.tensor.matmul(
                                spt[:, h, :],
                                NT[:, ob, j * P:(j + 1) * P],
                                Gp[:, ob, :],
                                start=(ob == 0), stop=(ob == DB - 1),
                            )
                    pt = ptp.tile([P, 2, QC], BF16, tag="pt", name="pt")
                    for h in range(2):
                        j = 2 * jj + h
                        nc.scalar.activation(pt[:, h, :], spt[:, h, :],
                                             mybir.ActivationFunctionType.Exp,
                                             bias=w_col[:, j:j + 1])
                    pts[jj] = pt
                    if jj == 0:
                        nc.gpsimd.tensor_copy(racc2, pt)
                    else:
                        nc.gpsimd.tensor_add(racc2, racc2, pt)
                    if c == 0:
                        # pipeline the remaining one-time builds into chunk 0
                        if jj < 2:
                            emit_nt_build(NZ_MAIN + 2 * jj, pool=mp)
                            emit_nt_build(NZ_MAIN + 2 * jj + 1, pool=mp)
                        else:
                            emit_wvt_build(jj - 2)
                if jj > 0:
                    zjj = jj - 1
                    pt = pts.pop(zjj)
                    for h in range(2):
                        j = 2 * zjj + h
                        for db in range(DB):
                            # start only on the even db of each 2KB PSUM zero
                            # region: its start marks the whole region, and
                            # the odd db's first write lands as overwrite
                            # (a second start would re-mark the even db's
                            # fresh j=0 data as pending-zero and lose it)
                            nzsrc = (Nz[:, j, db * P:(db + 1) * P]
                                     if j < NZ_MAIN else
                                     Nz_tail[:, j - NZ_MAIN, db * P:(db + 1) * P])
                            nc.tensor.matmul(
                                zp[:, db, :],
                                nzsrc,
                                pt[:, h, :],
                                start=(j == 0 and db % 2 == 0),
                                stop=(j == KVB - 1),
                                skip_group_check=True,
                            )

            if c == 0:
                emit_wvt_build(6)
                emit_wvt_build(7)

            # ---- Z psum -> sbuf bf16: 4-way split across DVE/Act so each
            # piece lands fast and the O matmuls / next chunk unblock early ----
            Zs = chk.tile([P, DB, QC], BF16, tag="zs", name="Zs")
            nc.vector.tensor_copy(Zs[:, 0:4, 0:P], zp[:, 0:4, 0:P])
            nc.scalar.activation(Zs[:, 4:8, 0:P], zp[:, 4:8, 0:P],
                                 mybir.ActivationFunctionType.Copy)
            nc.vector.tensor_copy(Zs[:, 0:4, P:QC], zp[:, 0:4, P:QC])
            nc.scalar.activation(Zs[:, 4:8, P:QC], zp[:, 4:8, P:QC],
                                 mybir.ActivationFunctionType.Copy)

            TT_next = Gp_next = None
            if c + 1 < NCHUNK:
                TT_next = emit_tt(c + 1)

            nc.gpsimd.tensor_add(racc, racc2[:, 0, :], racc2[:, 1, :])

            # ---- r chain: (PE transpose, DVE reduce, recip) ----
            rr_cols = []
            for qb in range(QC // P):
                tp = mp.tile([P, 512], F32R, tag="mp", name="rtp")
                nc.tensor.transpose(tp[:, 0:P],
                                    racc[:, qb * P:(qb + 1) * P], identR)
                rcol = smal.tile([P, 1], F32, tag="rcol", name="rcol")
                nc.vector.reduce_sum(out=rcol, in_=tp[:, 0:P].bitcast(F32),
                                     axis=mybir.AxisListType.X)
                rr = smal.tile([P, 1], F32, tag="rr", name="rr")
                nc.vector.reciprocal(rr, rcol)
                rr_cols.append(rr)

            if c + 1 < NCHUNK:
                Gp_next = emit_gp(TT_next)

            # ---- O = (Z^T.T @ Wv^T) * (1/r) + bv ----
            for qb in range(QC // P):
                ot = osb.tile([P, D], F32, tag="ot", name="ot")
                for ec in range(2):
                    op_ps = mp.tile([P, 512], F32, tag="mp", name="op_ps")
                    for db in range(DB):
                        nc.tensor.matmul(
                            op_ps,
                            Zs[:, db, qb * P:(qb + 1) * P],
                            WvT[:, db, ec * 512:(ec + 1) * 512],
                            start=(db == 0), stop=(db == DB - 1),
                        )
                    nc.vector.tensor_scalar_mul(
                        ot[:, ec * 512:(ec + 1) * 512], op_ps, rr_cols[qb])
                    nc.gpsimd.tensor_add(
                        ot[:, ec * 512:(ec + 1) * 512],
                        ot[:, ec * 512:(ec + 1) * 512],
                        bv_bc[:, ec * 512:(ec + 1) * 512])
                    nc.scalar.dma_start(
                        outp[q0 + qb * P:q0 + (qb + 1) * P,
                             ec * 512:(ec + 1) * 512],
                        ot[:, ec * 512:(ec + 1) * 512])
            TT, Gp = TT_next, Gp_next


def _get_nc():
    if "nc" not in _CACHE:
        _CACHE["nc"] = _build()
    return _CACHE["nc"]


def kernel(**inputs):
    inp = {k: np.ascontiguousarray(np.asarray(v, dtype=np.float32))
           for k, v in inputs.items()}
    nc = _get_nc()
    in_maps = [
        {
            "target": inp["target"][b],
            "non_target": inp["non_target"][b],
            "Wq": inp["Wq"], "Wk": inp["Wk"], "Wv": inp["Wv"],
            "bq": inp["bq"], "bv": inp["bv"],
        }
        for b in range(NCORES)
    ]
    res = run_bass_kernel_spmd(nc, in_maps, list(range(NCORES)))
    _CACHE["last_result"] = res
    out = np.stack([res.results[b]["out"] for b in range(NCORES)], axis=0)
    return out
